# revision 19
# baseline (speedup 1.0000x reference)
"""DCNv2 (deformable conv) Trainium2 kernel.

Strategy (data-parallel over batch, one sample per NeuronCore):
  Host: pad x to 128x128, build a channels-last "quad image" where row
  (y,x) holds the 2x2 bilinear corner patch for all 128 channels
  (fp16, 1KB rows). Compute int16 gather indices and the 4 bilinear
  corner weights per (tap k, pixel) from `offset`.
  Device, per (stripe of 2304 pixels) x (9 taps):
    dma_gather (SWDGE)  -> G [128 pix, 18 blk, 4*128] fp16. One call per
      (tap, stripe), rotated over the 4 SWDGE queues so descriptor
      generation runs on all four Q7 core-pairs concurrently (~4x).
    4x DVE mul (in-place, weights broadcast along channels via dup-pair
      stride-0 APs) + 2 DVE adds -> h [pix, c] per tap
    PE: per 128-px block, one transpose-matmul (h vs identity) into a
      4-block-packed PSUM bank -> S[c, pix]
    ACT: batched PSUM->SBUF fp16 copies (512 px at a time)
    9-tap GEMM accumulating in PSUM fp32 -> out [o, pix]
"""

import numpy as np

import concourse.mybir as mybir
import concourse.tile as tile
from concourse import bacc, bass_utils, library_config

P = 128
B, C, H, W, KK = 8, 128, 96, 96, 3
HW = H * W                  # 9216
NK = KK * KK                # 9
PAD = 16
HP = WP = 128
NROW = HP * WP              # 16384 quad-image rows
ELEM = 4 * C                # 512 fp16 elems per quad row (1KB)
NSTR = 4                    # pixel stripes
SPIX = HW // NSTR           # 2304 pixels per stripe
NBLK = SPIX // P            # 18
NCH = 6                     # GEMM n-chunks per stripe
CHW = SPIX // NCH           # 384
ICOL = SPIX // 16           # 144 wrapped-idx columns per (k, stripe)
NGRP = 5                    # PE/ACT block groups per (k, stripe): 4+4+4+4+2

F16, F32, I16 = mybir.dt.float16, mybir.dt.float32, mybir.dt.int16

TRACE = False               # set by test harness to capture a profile
LAST_RESULTS = None

_CACHE = {}


def _build():
    key = "nc"
    if key in _CACHE:
        return _CACHE[key]
    nc = bacc.Bacc("TRN2", target_bir_lowering=False, debug=False,
                   enable_asserts=False, num_swdge_queues=4)
    xq_d = nc.dram_tensor("xq", [NROW, ELEM], F16, kind="ExternalInput")
    idx_d = nc.dram_tensor("idx", [P, NK * NSTR * ICOL], I16,
                           kind="ExternalInput")
    wts_d = nc.dram_tensor("wts", [P, NK * NSTR * NBLK * 8], F16,
                           kind="ExternalInput")
    w2_d = nc.dram_tensor("w2", [P, NK * P], F16, kind="ExternalInput")
    id_d = nc.dram_tensor("idm", [P, P], F16, kind="ExternalInput")
    out_d = nc.dram_tensor("out", [P, HW], F32, kind="ExternalOutput")

    with (
        tile.TileContext(nc) as tc,
        tc.tile_pool(name="const", bufs=1) as const_p,
        tc.tile_pool(name="g", bufs=4) as g_p,
        tc.tile_pool(name="h", bufs=2) as h_p,
        tc.tile_pool(name="h2", bufs=2) as h2_p,
        tc.tile_pool(name="ssb", bufs=NK + 2) as ssb_p,
        tc.tile_pool(name="ob", bufs=2) as out_p,
        tc.tile_pool(name="tp", bufs=5, space="PSUM") as tp_p,
        tc.tile_pool(name="mm", bufs=2, space="PSUM") as mm_p,
    ):
        nc.gpsimd.load_library(library_config.mlp)
        ident = const_p.tile([P, P], F16)
        nc.sync.dma_start(ident[:], id_d[:])
        idx_sb = const_p.tile([P, NK, NSTR, ICOL], I16)
        nc.sync.dma_start(idx_sb[:], idx_d[:])
        wts_sb = const_p.tile([P, NK, NSTR, NBLK, 4, 2], F16)
        nc.sync.dma_start(wts_sb[:], wts_d[:])
        w2_sb = const_p.tile([P, NK, P], F16)
        nc.sync.dma_start(w2_sb[:], w2_d[:])

        for s in range(NSTR):
            ssb = []
            for k in range(NK):
                g_t = g_p.tile([P, NBLK, ELEM], F16)
                for hf in range(3):
                    nc.gpsimd.dma_gather(
                        g_t[:, hf * (NBLK // 3):(hf + 1) * (NBLK // 3), :],
                        xq_d[:],
                        idx_sb[:, k, s, hf * (ICOL // 3):(hf + 1) * (ICOL // 3)],
                        SPIX // 3, SPIX // 3, ELEM,
                        single_packet=False,
                        queue_num=(3 * (s * NK + k) + hf) % 4)
                # weighted corners: one in-place mul, weights broadcast
                # along c via dup-pair stride-0 AP
                v = g_t[:].rearrange("p b (j r d) -> p b j r d", j=4, d=2)
                w_ap = wts_sb[:, k, s, :, :, None, :].to_broadcast(
                    [P, NBLK, 4, P // 2, 2])
                nc.vector.tensor_tensor(out=v, in0=v, in1=w_ap,
                                        op=mybir.AluOpType.mult)
                # bilinear adds on DVE: (g0w+g2w, g1w+g3w) then final sum
                h2_t = h2_p.tile([P, NBLK, 2, P], F16)
                nc.vector.tensor_add(
                    out=h2_t[:],
                    in0=g_t[:, :, 0:2 * P].rearrange(
                        "p b (e r) -> p b e r", e=2),
                    in1=g_t[:, :, 2 * P:4 * P].rearrange(
                        "p b (e r) -> p b e r", e=2))
                h_t = h_p.tile([P, NBLK, P], F16)
                nc.vector.tensor_add(out=h_t[:], in0=h2_t[:, :, 0],
                                     in1=h2_t[:, :, 1])
                s_sb = ssb_p.tile([P, SPIX], F16)
                for grp in range(NGRP):
                    b0 = grp * 4
                    nb = min(4, NBLK - b0)
                    tp_t = tp_p.tile([P, 4, P], F16)
                    for bi in range(nb):
                        nc.tensor.matmul(out=tp_t[:, bi], lhsT=h_t[:, b0 + bi],
                                         rhs=ident[:], start=True, stop=True,
                                         is_transpose=True)
                    nc.scalar.copy(out=s_sb[:, b0 * P:(b0 + nb) * P],
                                   in_=tp_t[:, :nb].rearrange("p a b -> p (a b)"))
                ssb.append(s_sb)
            for n in range(NCH):
                mm_t = mm_p.tile([P, CHW], F32)
                for k in range(NK):
                    nc.tensor.matmul(out=mm_t[:], lhsT=w2_sb[:, k],
                                     rhs=ssb[k][:, n * CHW:(n + 1) * CHW],
                                     start=(k == 0), stop=(k == NK - 1))
                o_sb = out_p.tile([P, CHW], F32)
                nc.scalar.copy(out=o_sb[:], in_=mm_t[:])
                nc.sync.dma_start(
                    out_d[:, s * SPIX + n * CHW: s * SPIX + (n + 1) * CHW],
                    o_sb[:])
    nc.compile()
    _CACHE[key] = nc
    return nc


def _host_prep(x, offset, weight):
    x = np.asarray(x, dtype=np.float32)
    offset = np.asarray(offset, dtype=np.float32)
    weight = np.asarray(weight, dtype=np.float32)

    # quad image [B, NROW, 4*C] fp16, zero padded
    xt = np.zeros((B, HP + 1, WP + 1, C), np.float16)
    xt[:, PAD:PAD + H, PAD:PAD + W, :] = np.transpose(
        x, (0, 2, 3, 1)).astype(np.float16)
    quad = np.stack([xt[:, :HP, :WP], xt[:, :HP, 1:],
                     xt[:, 1:, :WP], xt[:, 1:, 1:]], axis=3)
    xq = np.ascontiguousarray(quad.reshape(B, NROW, ELEM))

    # sampling positions (float32, matching the reference exactly)
    off = offset.reshape(B, NK, 2, H, W)
    oy = np.arange(H, dtype=np.float32).reshape(1, 1, H, 1)
    ox = np.arange(W, dtype=np.float32).reshape(1, 1, 1, W)
    kh = (np.arange(NK) // KK).astype(np.float32).reshape(1, NK, 1, 1)
    kw = (np.arange(NK) % KK).astype(np.float32).reshape(1, NK, 1, 1)
    py = oy - 1.0 + kh + off[:, :, 0]
    px = ox - 1.0 + kw + off[:, :, 1]
    y0 = np.floor(py)
    x0 = np.floor(px)
    dy = py - y0
    dx = px - x0
    ry = np.clip(y0.astype(np.int32) + PAD, 0, HP - 2)
    rx = np.clip(x0.astype(np.int32) + PAD, 0, WP - 2)
    idx = (ry * WP + rx).astype(np.int16)                    # [B,NK,H,W]

    # wrapped gather indices: [B, 128, NK*NSTR*ICOL]
    idxf = idx.reshape(B, NK, NSTR, ICOL, 16)
    idxw = idxf.transpose(0, 1, 2, 4, 3)                     # [B,NK,NSTR,16,ICOL]
    idxw = np.broadcast_to(idxw[:, :, :, None],
                           (B, NK, NSTR, 8, 16, ICOL))
    idx_host = np.ascontiguousarray(
        idxw.transpose(0, 3, 4, 1, 2, 5).reshape(B, P, NK * NSTR * ICOL))

    # corner weights [B, 128, NK*NSTR*NBLK*4*2] fp16 (dup pairs)
    w4 = np.stack([(1 - dy) * (1 - dx), (1 - dy) * dx,
                   dy * (1 - dx), dy * dx], axis=-1).astype(np.float16)
    w5 = w4.reshape(B, NK, NSTR, NBLK, P, 4)
    w_host = w5.transpose(0, 4, 1, 2, 3, 5)                  # [B,P,NK,NSTR,NBLK,4]
    w_host = np.ascontiguousarray(
        np.repeat(w_host[..., None], 2, axis=-1).reshape(
            B, P, NK * NSTR * NBLK * 8))

    # GEMM weights: lhsT per tap = W_k^T [c, o]
    w2h = weight.reshape(C, C, NK).transpose(2, 1, 0).astype(np.float16)
    w2_host = np.ascontiguousarray(w2h.transpose(1, 0, 2).reshape(P, NK * P))
    return xq, idx_host, w_host, w2_host



_EYE = np.eye(P, dtype=np.float16)


def kernel(x, offset, weight):
    global LAST_RESULTS
    nc = _build()
    xq, idx_host, w_host, w2_host = _host_prep(x, offset, weight)
    in_maps = [
        {"xq": xq[b], "idx": idx_host[b], "wts": w_host[b], "w2": w2_host,
         "idm": _EYE}
        for b in range(B)
    ]
    res = bass_utils.run_bass_kernel_spmd(
        nc, in_maps, core_ids=list(range(B)), trace=TRACE)
    LAST_RESULTS = res
    out = np.stack([res.results[b]["out"] for b in range(B)])
    return out.reshape(B, C, H, W).astype(np.float32)


# revision 20
# speedup vs baseline: 1.0451x; 1.0451x over previous
"""DCNv2 (deformable conv) Trainium2 kernel.

Strategy (data-parallel over batch, one sample per NeuronCore):
  Host: pad x to 128x128, build a channels-last "quad image" where row
  (y,x) holds the 2x2 bilinear corner patch for all 128 channels
  (fp16, 1KB rows). Compute int16 gather indices and the 4 bilinear
  corner weights per (tap k, pixel) from `offset`.
  Device, per (stripe of 2304 pixels) x (9 taps):
    dma_gather (SWDGE)  -> G [128 pix, 18 blk, 4*128] fp16. One call per
      (tap, stripe), rotated over the 4 SWDGE queues so descriptor
      generation runs on all four Q7 core-pairs concurrently (~4x).
    4x DVE mul (in-place, weights broadcast along channels via dup-pair
      stride-0 APs) + 2 DVE adds -> h [pix, c] per tap
    PE: per 128-px block, one transpose-matmul (h vs identity) into a
      4-block-packed PSUM bank -> S[c, pix]
    ACT: batched PSUM->SBUF fp16 copies (512 px at a time)
    9-tap GEMM accumulating in PSUM fp32 -> out [o, pix]
"""

import numpy as np

import concourse.mybir as mybir
import concourse.tile as tile
from concourse import bacc, bass_utils, library_config

P = 128
B, C, H, W, KK = 8, 128, 96, 96, 3
HW = H * W                  # 9216
NK = KK * KK                # 9
PAD = 16
HP = WP = 128
NROW = HP * WP              # 16384 quad-image rows
ELEM = 4 * C                # 512 fp16 elems per quad row (1KB)
NSTR = 4                    # pixel stripes
SPIX = HW // NSTR           # 2304 pixels per stripe
NBLK = SPIX // P            # 18
NCH = 6                     # GEMM n-chunks per stripe
CHW = SPIX // NCH           # 384
ICOL = SPIX // 16           # 144 wrapped-idx columns per (k, stripe)
NGRP = 5                    # PE/ACT block groups per (k, stripe): 4+4+4+4+2

F16, F32, I16 = mybir.dt.float16, mybir.dt.float32, mybir.dt.int16

TRACE = False               # set by test harness to capture a profile
LAST_RESULTS = None

_CACHE = {}


def _build():
    key = "nc"
    if key in _CACHE:
        return _CACHE[key]
    nc = bacc.Bacc("TRN2", target_bir_lowering=False, debug=False,
                   enable_asserts=False, num_swdge_queues=4)
    xq_d = nc.dram_tensor("xq", [NROW, ELEM], F16, kind="ExternalInput")
    idx_d = nc.dram_tensor("idx", [P, NK * NSTR * ICOL], I16,
                           kind="ExternalInput")
    wts_d = nc.dram_tensor("wts", [P, NK * NSTR * NBLK * 8], F16,
                           kind="ExternalInput")
    w2_d = nc.dram_tensor("w2", [P, NK * P], F16, kind="ExternalInput")
    id_d = nc.dram_tensor("idm", [P, P], F16, kind="ExternalInput")
    out_d = nc.dram_tensor("out", [P, HW], F32, kind="ExternalOutput")

    with (
        tile.TileContext(nc) as tc,
        tc.tile_pool(name="const", bufs=1) as const_p,
        tc.tile_pool(name="g", bufs=4) as g_p,
        tc.tile_pool(name="h", bufs=2) as h_p,
        tc.tile_pool(name="h2", bufs=2) as h2_p,
        tc.tile_pool(name="ssb", bufs=NK + 2) as ssb_p,
        tc.tile_pool(name="ob", bufs=2) as out_p,
        tc.tile_pool(name="tp", bufs=5, space="PSUM") as tp_p,
        tc.tile_pool(name="mm", bufs=2, space="PSUM") as mm_p,
    ):
        nc.gpsimd.load_library(library_config.mlp)
        ident = const_p.tile([P, P], F16)
        nc.sync.dma_start(ident[:], id_d[:])
        idx_sb = const_p.tile([P, NK, NSTR, ICOL], I16)
        nc.sync.dma_start(idx_sb[:], idx_d[:])
        wts_sb = const_p.tile([P, NK, NSTR, NBLK, 4, 2], F16)
        nc.sync.dma_start(wts_sb[:], wts_d[:])
        w2_sb = const_p.tile([P, NK, P], F16)
        nc.sync.dma_start(w2_sb[:], w2_d[:])

        for s in range(NSTR):
            ssb = []
            for k in range(NK):
                g_t = g_p.tile([P, NBLK, ELEM], F16)
                for hf in range(3):
                    nc.gpsimd.dma_gather(
                        g_t[:, hf * (NBLK // 3):(hf + 1) * (NBLK // 3), :],
                        xq_d[:],
                        idx_sb[:, k, s, hf * (ICOL // 3):(hf + 1) * (ICOL // 3)],
                        SPIX // 3, SPIX // 3, ELEM,
                        single_packet=False,
                        queue_num=(3 * (s * NK + k) + hf) % 4)
                # weighted corners: in-place mul, weight broadcast along c
                for c_ in range(4):
                    v = g_t[:, :, c_ * P:(c_ + 1) * P].rearrange(
                        "p b (r d) -> p b r d", d=2)
                    w_ap = wts_sb[:, k, s, :, c_:c_ + 1, :].to_broadcast(
                        [P, NBLK, P // 2, 2])
                    nc.vector.tensor_tensor(out=v, in0=v, in1=w_ap,
                                            op=mybir.AluOpType.mult)
                # bilinear adds on DVE: (g0w+g2w, g1w+g3w) then final sum
                h2_t = h2_p.tile([P, NBLK, 2, P + 32], F16)
                nc.vector.tensor_add(
                    out=h2_t[:, :, :, :P],
                    in0=g_t[:, :, 0:2 * P].rearrange(
                        "p b (e r) -> p b e r", e=2),
                    in1=g_t[:, :, 2 * P:4 * P].rearrange(
                        "p b (e r) -> p b e r", e=2))
                h_t = h_p.tile([P, NBLK, P], F16)
                nc.vector.tensor_add(out=h_t[:], in0=h2_t[:, :, 0, :P],
                                     in1=h2_t[:, :, 1, :P])
                s_sb = ssb_p.tile([P, SPIX], F16)
                for grp in range(NGRP):
                    b0 = grp * 4
                    nb = min(4, NBLK - b0)
                    tp_t = tp_p.tile([P, 4, P], F16)
                    for bi in range(nb):
                        nc.tensor.matmul(out=tp_t[:, bi], lhsT=h_t[:, b0 + bi],
                                         rhs=ident[:], start=True, stop=True,
                                         is_transpose=True)
                    nc.scalar.copy(out=s_sb[:, b0 * P:(b0 + nb) * P],
                                   in_=tp_t[:, :nb].rearrange("p a b -> p (a b)"))
                ssb.append(s_sb)
            for n in range(NCH):
                mm_t = mm_p.tile([P, CHW], F32)
                for k in range(NK):
                    nc.tensor.matmul(out=mm_t[:], lhsT=w2_sb[:, k],
                                     rhs=ssb[k][:, n * CHW:(n + 1) * CHW],
                                     start=(k == 0), stop=(k == NK - 1))
                o_sb = out_p.tile([P, CHW], F32)
                nc.scalar.copy(out=o_sb[:], in_=mm_t[:])
                nc.sync.dma_start(
                    out_d[:, s * SPIX + n * CHW: s * SPIX + (n + 1) * CHW],
                    o_sb[:])
    nc.compile()
    _CACHE[key] = nc
    return nc


def _host_prep(x, offset, weight):
    x = np.asarray(x, dtype=np.float32)
    offset = np.asarray(offset, dtype=np.float32)
    weight = np.asarray(weight, dtype=np.float32)

    # quad image [B, NROW, 4*C] fp16, zero padded
    xt = np.zeros((B, HP + 1, WP + 1, C), np.float16)
    xt[:, PAD:PAD + H, PAD:PAD + W, :] = np.transpose(
        x, (0, 2, 3, 1)).astype(np.float16)
    quad = np.stack([xt[:, :HP, :WP], xt[:, :HP, 1:],
                     xt[:, 1:, :WP], xt[:, 1:, 1:]], axis=3)
    xq = np.ascontiguousarray(quad.reshape(B, NROW, ELEM))

    # sampling positions (float32, matching the reference exactly)
    off = offset.reshape(B, NK, 2, H, W)
    oy = np.arange(H, dtype=np.float32).reshape(1, 1, H, 1)
    ox = np.arange(W, dtype=np.float32).reshape(1, 1, 1, W)
    kh = (np.arange(NK) // KK).astype(np.float32).reshape(1, NK, 1, 1)
    kw = (np.arange(NK) % KK).astype(np.float32).reshape(1, NK, 1, 1)
    py = oy - 1.0 + kh + off[:, :, 0]
    px = ox - 1.0 + kw + off[:, :, 1]
    y0 = np.floor(py)
    x0 = np.floor(px)
    dy = py - y0
    dx = px - x0
    ry = np.clip(y0.astype(np.int32) + PAD, 0, HP - 2)
    rx = np.clip(x0.astype(np.int32) + PAD, 0, WP - 2)
    idx = (ry * WP + rx).astype(np.int16)                    # [B,NK,H,W]

    # wrapped gather indices: [B, 128, NK*NSTR*ICOL]
    idxf = idx.reshape(B, NK, NSTR, ICOL, 16)
    idxw = idxf.transpose(0, 1, 2, 4, 3)                     # [B,NK,NSTR,16,ICOL]
    idxw = np.broadcast_to(idxw[:, :, :, None],
                           (B, NK, NSTR, 8, 16, ICOL))
    idx_host = np.ascontiguousarray(
        idxw.transpose(0, 3, 4, 1, 2, 5).reshape(B, P, NK * NSTR * ICOL))

    # corner weights [B, 128, NK*NSTR*NBLK*4*2] fp16 (dup pairs)
    w4 = np.stack([(1 - dy) * (1 - dx), (1 - dy) * dx,
                   dy * (1 - dx), dy * dx], axis=-1).astype(np.float16)
    w5 = w4.reshape(B, NK, NSTR, NBLK, P, 4)
    w_host = w5.transpose(0, 4, 1, 2, 3, 5)                  # [B,P,NK,NSTR,NBLK,4]
    w_host = np.ascontiguousarray(
        np.repeat(w_host[..., None], 2, axis=-1).reshape(
            B, P, NK * NSTR * NBLK * 8))

    # GEMM weights: lhsT per tap = W_k^T [c, o]
    w2h = weight.reshape(C, C, NK).transpose(2, 1, 0).astype(np.float16)
    w2_host = np.ascontiguousarray(w2h.transpose(1, 0, 2).reshape(P, NK * P))
    return xq, idx_host, w_host, w2_host



_EYE = np.eye(P, dtype=np.float16)


def kernel(x, offset, weight):
    global LAST_RESULTS
    nc = _build()
    xq, idx_host, w_host, w2_host = _host_prep(x, offset, weight)
    in_maps = [
        {"xq": xq[b], "idx": idx_host[b], "wts": w_host[b], "w2": w2_host,
         "idm": _EYE}
        for b in range(B)
    ]
    res = bass_utils.run_bass_kernel_spmd(
        nc, in_maps, core_ids=list(range(B)), trace=TRACE)
    LAST_RESULTS = res
    out = np.stack([res.results[b]["out"] for b in range(B)])
    return out.reshape(B, C, H, W).astype(np.float32)


# revision 21
# speedup vs baseline: 1.0462x; 1.0010x over previous
"""DCNv2 (deformable conv) Trainium2 kernel.

Strategy (data-parallel over batch, one sample per NeuronCore):
  Host: pad x to 128x128, build a channels-last "quad image" where row
  (y,x) holds the 2x2 bilinear corner patch for all 128 channels
  (fp16, 1KB rows). Compute int16 gather indices and the 4 bilinear
  corner weights per (tap k, pixel) from `offset`.
  Device, per (stripe of 2304 pixels) x (9 taps):
    dma_gather (SWDGE)  -> G [128 pix, 18 blk, 4*128] fp16. One call per
      (tap, stripe), rotated over the 4 SWDGE queues so descriptor
      generation runs on all four Q7 core-pairs concurrently (~4x).
    4x DVE mul (in-place, weights broadcast along channels via dup-pair
      stride-0 APs) + 2 DVE adds -> h [pix, c] per tap
    PE: per 128-px block, one transpose-matmul (h vs identity) into a
      4-block-packed PSUM bank -> S[c, pix]
    ACT: batched PSUM->SBUF fp16 copies (512 px at a time)
    9-tap GEMM accumulating in PSUM fp32 -> out [o, pix]
"""

import numpy as np

import concourse.mybir as mybir
import concourse.tile as tile
from concourse import bacc, bass_utils, library_config

P = 128
B, C, H, W, KK = 8, 128, 96, 96, 3
HW = H * W                  # 9216
NK = KK * KK                # 9
PAD = 16
HP = WP = 128
NROW = HP * WP              # 16384 quad-image rows
ELEM = 4 * C                # 512 fp16 elems per quad row (1KB)
NSTR = 4                    # pixel stripes
SPIX = HW // NSTR           # 2304 pixels per stripe
NBLK = SPIX // P            # 18
NCH = 6                     # GEMM n-chunks per stripe
CHW = SPIX // NCH           # 384
ICOL = SPIX // 16           # 144 wrapped-idx columns per (k, stripe)
NGRP = 5                    # PE/ACT block groups per (k, stripe): 4+4+4+4+2

F16, F32, I16 = mybir.dt.float16, mybir.dt.float32, mybir.dt.int16

TRACE = False               # set by test harness to capture a profile
LAST_RESULTS = None

_CACHE = {}


def _build():
    key = "nc"
    if key in _CACHE:
        return _CACHE[key]
    nc = bacc.Bacc("TRN2", target_bir_lowering=False, debug=False,
                   enable_asserts=False, num_swdge_queues=4)
    xq_d = nc.dram_tensor("xq", [NROW, ELEM], F16, kind="ExternalInput")
    idx_d = nc.dram_tensor("idx", [P, NK * NSTR * ICOL], I16,
                           kind="ExternalInput")
    wts_d = nc.dram_tensor("wts", [P, NK * NSTR * NBLK * 8], F16,
                           kind="ExternalInput")
    w2_d = nc.dram_tensor("w2", [P, NK * P], F16, kind="ExternalInput")
    id_d = nc.dram_tensor("idm", [P, P], F16, kind="ExternalInput")
    out_d = nc.dram_tensor("out", [P, HW], F32, kind="ExternalOutput")

    with (
        tile.TileContext(nc) as tc,
        tc.tile_pool(name="const", bufs=1) as const_p,
        tc.tile_pool(name="g", bufs=4) as g_p,
        tc.tile_pool(name="h", bufs=2) as h_p,
        tc.tile_pool(name="h2", bufs=2) as h2_p,
        tc.tile_pool(name="ssb", bufs=NK + 2) as ssb_p,
        tc.tile_pool(name="ob", bufs=2) as out_p,
        tc.tile_pool(name="tp", bufs=5, space="PSUM") as tp_p,
        tc.tile_pool(name="mm", bufs=2, space="PSUM") as mm_p,
    ):
        nc.gpsimd.load_library(library_config.mlp)
        ident = const_p.tile([P, P], F16)
        nc.sync.dma_start(ident[:], id_d[:])
        idx_sb = const_p.tile([P, NK, NSTR, ICOL], I16)
        nc.sync.dma_start(idx_sb[:], idx_d[:])
        wts_sb = const_p.tile([P, NK, NSTR, NBLK, 4, 2], F16)
        nc.sync.dma_start(wts_sb[:], wts_d[:])
        w2_sb = const_p.tile([P, NK, P], F16)
        nc.sync.dma_start(w2_sb[:], w2_d[:])

        for s in range(NSTR):
            ssb = []
            for k in range(NK):
                g_t = g_p.tile([P, NBLK, ELEM], F16)
                for hf in range(3):
                    nc.gpsimd.dma_gather(
                        g_t[:, hf * (NBLK // 3):(hf + 1) * (NBLK // 3), :],
                        xq_d[:],
                        idx_sb[:, k, s, hf * (ICOL // 3):(hf + 1) * (ICOL // 3)],
                        SPIX // 3, SPIX // 3, ELEM,
                        single_packet=False,
                        queue_num=(3 * (s * NK + k) + hf) % 4)
                # weighted corners: in-place mul, weight broadcast along c
                for c_ in range(4):
                    v = g_t[:, :, c_ * P:(c_ + 1) * P].rearrange(
                        "p b (r d) -> p b r d", d=2)
                    w_ap = wts_sb[:, k, s, :, c_:c_ + 1, :].to_broadcast(
                        [P, NBLK, P // 2, 2])
                    nc.vector.tensor_tensor(out=v, in0=v, in1=w_ap,
                                            op=mybir.AluOpType.mult)
                # bilinear adds on DVE: (g0w+g2w, g1w+g3w) then final sum
                h2_t = h2_p.tile([P, NBLK, 2, P], F16)
                nc.vector.tensor_add(
                    out=h2_t[:],
                    in0=g_t[:, :, 0:2 * P].rearrange(
                        "p b (e r) -> p b e r", e=2),
                    in1=g_t[:, :, 2 * P:4 * P].rearrange(
                        "p b (e r) -> p b e r", e=2))
                h_t = h_p.tile([P, NBLK, P], F16)
                nc.vector.tensor_add(out=h_t[:], in0=h2_t[:, :, 0],
                                     in1=h2_t[:, :, 1])
                s_sb = ssb_p.tile([P, SPIX], F16)
                for grp in range(NGRP):
                    b0 = grp * 4
                    nb = min(4, NBLK - b0)
                    tp_t = tp_p.tile([P, 4, P], F16)
                    for bi in range(nb):
                        nc.tensor.matmul(out=tp_t[:, bi], lhsT=h_t[:, b0 + bi],
                                         rhs=ident[:], start=True, stop=True,
                                         is_transpose=True)
                    nc.scalar.copy(out=s_sb[:, b0 * P:(b0 + nb) * P],
                                   in_=tp_t[:, :nb].rearrange("p a b -> p (a b)"))
                ssb.append(s_sb)
            for n in range(NCH):
                mm_t = mm_p.tile([P, CHW], F32)
                for k in range(NK):
                    nc.tensor.matmul(out=mm_t[:], lhsT=w2_sb[:, k],
                                     rhs=ssb[k][:, n * CHW:(n + 1) * CHW],
                                     start=(k == 0), stop=(k == NK - 1))
                o_sb = out_p.tile([P, CHW], F32)
                nc.scalar.copy(out=o_sb[:], in_=mm_t[:])
                nc.sync.dma_start(
                    out_d[:, s * SPIX + n * CHW: s * SPIX + (n + 1) * CHW],
                    o_sb[:])
    nc.compile()
    _CACHE[key] = nc
    return nc


def _host_prep(x, offset, weight):
    x = np.asarray(x, dtype=np.float32)
    offset = np.asarray(offset, dtype=np.float32)
    weight = np.asarray(weight, dtype=np.float32)

    # quad image [B, NROW, 4*C] fp16, zero padded
    xt = np.zeros((B, HP + 1, WP + 1, C), np.float16)
    xt[:, PAD:PAD + H, PAD:PAD + W, :] = np.transpose(
        x, (0, 2, 3, 1)).astype(np.float16)
    quad = np.stack([xt[:, :HP, :WP], xt[:, :HP, 1:],
                     xt[:, 1:, :WP], xt[:, 1:, 1:]], axis=3)
    xq = np.ascontiguousarray(quad.reshape(B, NROW, ELEM))

    # sampling positions (float32, matching the reference exactly)
    off = offset.reshape(B, NK, 2, H, W)
    oy = np.arange(H, dtype=np.float32).reshape(1, 1, H, 1)
    ox = np.arange(W, dtype=np.float32).reshape(1, 1, 1, W)
    kh = (np.arange(NK) // KK).astype(np.float32).reshape(1, NK, 1, 1)
    kw = (np.arange(NK) % KK).astype(np.float32).reshape(1, NK, 1, 1)
    py = oy - 1.0 + kh + off[:, :, 0]
    px = ox - 1.0 + kw + off[:, :, 1]
    y0 = np.floor(py)
    x0 = np.floor(px)
    dy = py - y0
    dx = px - x0
    ry = np.clip(y0.astype(np.int32) + PAD, 0, HP - 2)
    rx = np.clip(x0.astype(np.int32) + PAD, 0, WP - 2)
    idx = (ry * WP + rx).astype(np.int16)                    # [B,NK,H,W]

    # wrapped gather indices: [B, 128, NK*NSTR*ICOL]
    idxf = idx.reshape(B, NK, NSTR, ICOL, 16)
    idxw = idxf.transpose(0, 1, 2, 4, 3)                     # [B,NK,NSTR,16,ICOL]
    idxw = np.broadcast_to(idxw[:, :, :, None],
                           (B, NK, NSTR, 8, 16, ICOL))
    idx_host = np.ascontiguousarray(
        idxw.transpose(0, 3, 4, 1, 2, 5).reshape(B, P, NK * NSTR * ICOL))

    # corner weights [B, 128, NK*NSTR*NBLK*4*2] fp16 (dup pairs)
    w4 = np.stack([(1 - dy) * (1 - dx), (1 - dy) * dx,
                   dy * (1 - dx), dy * dx], axis=-1).astype(np.float16)
    w5 = w4.reshape(B, NK, NSTR, NBLK, P, 4)
    w_host = w5.transpose(0, 4, 1, 2, 3, 5)                  # [B,P,NK,NSTR,NBLK,4]
    w_host = np.ascontiguousarray(
        np.repeat(w_host[..., None], 2, axis=-1).reshape(
            B, P, NK * NSTR * NBLK * 8))

    # GEMM weights: lhsT per tap = W_k^T [c, o]
    w2h = weight.reshape(C, C, NK).transpose(2, 1, 0).astype(np.float16)
    w2_host = np.ascontiguousarray(w2h.transpose(1, 0, 2).reshape(P, NK * P))
    return xq, idx_host, w_host, w2_host



_EYE = np.eye(P, dtype=np.float16)


def kernel(x, offset, weight):
    global LAST_RESULTS
    nc = _build()
    xq, idx_host, w_host, w2_host = _host_prep(x, offset, weight)
    in_maps = [
        {"xq": xq[b], "idx": idx_host[b], "wts": w_host[b], "w2": w2_host,
         "idm": _EYE}
        for b in range(B)
    ]
    res = bass_utils.run_bass_kernel_spmd(
        nc, in_maps, core_ids=list(range(B)), trace=TRACE)
    LAST_RESULTS = res
    out = np.stack([res.results[b]["out"] for b in range(B)])
    return out.reshape(B, C, H, W).astype(np.float32)


# revision 23
# speedup vs baseline: 1.0467x; 1.0006x over previous
"""DCNv2 (deformable conv) Trainium2 kernel.

Strategy (data-parallel over batch, one sample per NeuronCore):
  Host: pad x to 128x128, build a channels-last "quad image" where row
  (y,x) holds the 2x2 bilinear corner patch for all 128 channels
  (fp16, 1KB rows). Compute int16 gather indices and the 4 bilinear
  corner weights per (tap k, pixel) from `offset`.
  Device, per (stripe of 2304 pixels) x (9 taps):
    dma_gather (SWDGE)  -> G [128 pix, 18 blk, 4*128] fp16. Three 768-idx
      calls per (tap, stripe) rotated over the 4 SWDGE queues: queue_num
      selects the Q7 core pair (cpu_id/2 == queue_num in dma_gather.cpp),
      so descriptor generation runs on all four core pairs concurrently
      (~4x) and each tap's data lands with ~1/3 the latency.
    4x DVE mul (in-place, weights broadcast along channels via dup-pair
      stride-0 APs) + 2 DVE adds -> h [pix, c] per tap
    PE: per 128-px block, one transpose-mode matmul (h vs identity) into
      a 4-block-packed fp16 PSUM bank -> S[c, pix]
    ACT: batched PSUM->SBUF fp16 copies (512 px at a time)
    9-tap GEMM accumulating in PSUM fp32 -> out [o, pix]
"""

import numpy as np

import concourse.mybir as mybir
import concourse.tile as tile
from concourse import bacc, bass_utils, library_config

P = 128
B, C, H, W, KK = 8, 128, 96, 96, 3
HW = H * W                  # 9216
NK = KK * KK                # 9
PAD = 16
HP = WP = 128
NROW = HP * WP              # 16384 quad-image rows
ELEM = 4 * C                # 512 fp16 elems per quad row (1KB)
NSTR = 4                    # pixel stripes
SPIX = HW // NSTR           # 2304 pixels per stripe
NBLK = SPIX // P            # 18
NCH = 6                     # GEMM n-chunks per stripe
CHW = SPIX // NCH           # 384
ICOL = SPIX // 16           # 144 wrapped-idx columns per (k, stripe)
NGRP = 5                    # PE/ACT block groups per (k, stripe): 4+4+4+4+2

F16, F32, I16 = mybir.dt.float16, mybir.dt.float32, mybir.dt.int16

TRACE = False               # set by test harness to capture a profile
LAST_RESULTS = None

_CACHE = {}


def _build():
    key = "nc"
    if key in _CACHE:
        return _CACHE[key]
    nc = bacc.Bacc("TRN2", target_bir_lowering=False, debug=False,
                   enable_asserts=False, num_swdge_queues=4)
    xq_d = nc.dram_tensor("xq", [NROW, ELEM], F16, kind="ExternalInput")
    idx_d = nc.dram_tensor("idx", [P, NK * NSTR * ICOL], I16,
                           kind="ExternalInput")
    wts_d = nc.dram_tensor("wts", [P, NK * NSTR * NBLK * 8], F16,
                           kind="ExternalInput")
    w2_d = nc.dram_tensor("w2", [P, NK * P], F16, kind="ExternalInput")
    id_d = nc.dram_tensor("idm", [P, P], F16, kind="ExternalInput")
    out_d = nc.dram_tensor("out", [P, HW], F32, kind="ExternalOutput")

    with (
        tile.TileContext(nc) as tc,
        tc.tile_pool(name="const", bufs=1) as const_p,
        tc.tile_pool(name="g", bufs=4) as g_p,
        tc.tile_pool(name="h", bufs=3) as h_p,
        tc.tile_pool(name="h2", bufs=2) as h2_p,
        tc.tile_pool(name="ssb", bufs=NK + 2) as ssb_p,
        tc.tile_pool(name="ob", bufs=2) as out_p,
        tc.tile_pool(name="tp", bufs=5, space="PSUM") as tp_p,
        tc.tile_pool(name="mm", bufs=2, space="PSUM") as mm_p,
    ):
        nc.gpsimd.load_library(library_config.mlp)
        ident = const_p.tile([P, P], F16)
        nc.sync.dma_start(ident[:], id_d[:])
        idx_sb = const_p.tile([P, NK, NSTR, ICOL], I16)
        nc.sync.dma_start(idx_sb[:], idx_d[:])
        wts_sb = const_p.tile([P, NK, NSTR, NBLK, 4, 2], F16)
        nc.sync.dma_start(wts_sb[:], wts_d[:])
        w2_sb = const_p.tile([P, NK, P], F16)
        nc.sync.dma_start(w2_sb[:], w2_d[:])

        for s in range(NSTR):
            ssb = []
            for k in range(NK):
                g_t = g_p.tile([P, NBLK, ELEM], F16)
                for hf in range(3):
                    nc.gpsimd.dma_gather(
                        g_t[:, hf * (NBLK // 3):(hf + 1) * (NBLK // 3), :],
                        xq_d[:],
                        idx_sb[:, k, s, hf * (ICOL // 3):(hf + 1) * (ICOL // 3)],
                        SPIX // 3, SPIX // 3, ELEM,
                        single_packet=False,
                        queue_num=(3 * (s * NK + k) + hf) % 4)
                # weighted corners: in-place mul, weight broadcast along c
                for c_ in range(4):
                    v = g_t[:, :, c_ * P:(c_ + 1) * P].rearrange(
                        "p b (r d) -> p b r d", d=2)
                    w_ap = wts_sb[:, k, s, :, c_:c_ + 1, :].to_broadcast(
                        [P, NBLK, P // 2, 2])
                    nc.vector.tensor_tensor(out=v, in0=v, in1=w_ap,
                                            op=mybir.AluOpType.mult)
                # bilinear adds on DVE: (g0w+g2w, g1w+g3w) then final sum
                h2_t = h2_p.tile([P, NBLK, 2, P], F16)
                nc.vector.tensor_add(
                    out=h2_t[:],
                    in0=g_t[:, :, 0:2 * P].rearrange(
                        "p b (e r) -> p b e r", e=2),
                    in1=g_t[:, :, 2 * P:4 * P].rearrange(
                        "p b (e r) -> p b e r", e=2))
                h_t = h_p.tile([P, NBLK, P], F16)
                nc.vector.tensor_add(out=h_t[:], in0=h2_t[:, :, 0],
                                     in1=h2_t[:, :, 1])
                s_sb = ssb_p.tile([P, SPIX], F16)
                for grp in range(NGRP):
                    b0 = grp * 4
                    nb = min(4, NBLK - b0)
                    tp_t = tp_p.tile([P, 4, P], F16)
                    for bi in range(nb):
                        nc.tensor.matmul(out=tp_t[:, bi], lhsT=h_t[:, b0 + bi],
                                         rhs=ident[:], start=True, stop=True,
                                         is_transpose=True)
                    nc.scalar.copy(out=s_sb[:, b0 * P:(b0 + nb) * P],
                                   in_=tp_t[:, :nb].rearrange("p a b -> p (a b)"))
                ssb.append(s_sb)
            for n in range(NCH):
                mm_t = mm_p.tile([P, CHW], F32)
                for k in range(NK):
                    nc.tensor.matmul(out=mm_t[:], lhsT=w2_sb[:, k],
                                     rhs=ssb[k][:, n * CHW:(n + 1) * CHW],
                                     start=(k == 0), stop=(k == NK - 1))
                o_sb = out_p.tile([P, CHW], F32)
                nc.scalar.copy(out=o_sb[:], in_=mm_t[:])
                nc.sync.dma_start(
                    out_d[:, s * SPIX + n * CHW: s * SPIX + (n + 1) * CHW],
                    o_sb[:])
    nc.compile()
    _CACHE[key] = nc
    return nc


def _host_prep(x, offset, weight):
    x = np.asarray(x, dtype=np.float32)
    offset = np.asarray(offset, dtype=np.float32)
    weight = np.asarray(weight, dtype=np.float32)

    # quad image [B, NROW, 4*C] fp16, zero padded
    xt = np.zeros((B, HP + 1, WP + 1, C), np.float16)
    xt[:, PAD:PAD + H, PAD:PAD + W, :] = np.transpose(
        x, (0, 2, 3, 1)).astype(np.float16)
    quad = np.stack([xt[:, :HP, :WP], xt[:, :HP, 1:],
                     xt[:, 1:, :WP], xt[:, 1:, 1:]], axis=3)
    xq = np.ascontiguousarray(quad.reshape(B, NROW, ELEM))

    # sampling positions (float32, matching the reference exactly)
    off = offset.reshape(B, NK, 2, H, W)
    oy = np.arange(H, dtype=np.float32).reshape(1, 1, H, 1)
    ox = np.arange(W, dtype=np.float32).reshape(1, 1, 1, W)
    kh = (np.arange(NK) // KK).astype(np.float32).reshape(1, NK, 1, 1)
    kw = (np.arange(NK) % KK).astype(np.float32).reshape(1, NK, 1, 1)
    py = oy - 1.0 + kh + off[:, :, 0]
    px = ox - 1.0 + kw + off[:, :, 1]
    y0 = np.floor(py)
    x0 = np.floor(px)
    dy = py - y0
    dx = px - x0
    ry = np.clip(y0.astype(np.int32) + PAD, 0, HP - 2)
    rx = np.clip(x0.astype(np.int32) + PAD, 0, WP - 2)
    idx = (ry * WP + rx).astype(np.int16)                    # [B,NK,H,W]

    # wrapped gather indices: [B, 128, NK*NSTR*ICOL]
    idxf = idx.reshape(B, NK, NSTR, ICOL, 16)
    idxw = idxf.transpose(0, 1, 2, 4, 3)                     # [B,NK,NSTR,16,ICOL]
    idxw = np.broadcast_to(idxw[:, :, :, None],
                           (B, NK, NSTR, 8, 16, ICOL))
    idx_host = np.ascontiguousarray(
        idxw.transpose(0, 3, 4, 1, 2, 5).reshape(B, P, NK * NSTR * ICOL))

    # corner weights [B, 128, NK*NSTR*NBLK*4*2] fp16 (dup pairs)
    w4 = np.stack([(1 - dy) * (1 - dx), (1 - dy) * dx,
                   dy * (1 - dx), dy * dx], axis=-1).astype(np.float16)
    w5 = w4.reshape(B, NK, NSTR, NBLK, P, 4)
    w_host = w5.transpose(0, 4, 1, 2, 3, 5)                  # [B,P,NK,NSTR,NBLK,4]
    w_host = np.ascontiguousarray(
        np.repeat(w_host[..., None], 2, axis=-1).reshape(
            B, P, NK * NSTR * NBLK * 8))

    # GEMM weights: lhsT per tap = W_k^T [c, o]
    w2h = weight.reshape(C, C, NK).transpose(2, 1, 0).astype(np.float16)
    w2_host = np.ascontiguousarray(w2h.transpose(1, 0, 2).reshape(P, NK * P))
    return xq, idx_host, w_host, w2_host



_EYE = np.eye(P, dtype=np.float16)


def kernel(x, offset, weight):
    global LAST_RESULTS
    nc = _build()
    xq, idx_host, w_host, w2_host = _host_prep(x, offset, weight)
    in_maps = [
        {"xq": xq[b], "idx": idx_host[b], "wts": w_host[b], "w2": w2_host,
         "idm": _EYE}
        for b in range(B)
    ]
    res = bass_utils.run_bass_kernel_spmd(
        nc, in_maps, core_ids=list(range(B)), trace=TRACE)
    LAST_RESULTS = res
    out = np.stack([res.results[b]["out"] for b in range(B)])
    return out.reshape(B, C, H, W).astype(np.float32)
